# revision 1
# baseline (speedup 1.0000x reference)
"""Trainium2 Bass kernel for masked-mean-pooling + per-attribute softmax/loss.

Data-parallel over the batch: 8 NeuronCores x 2048 users each.
Per core: indirect-DMA gather of the embedding rows (50 per user) into a
[user, 50*64] SBUF layout, DVE reduction over the history dim, PE transpose +
tiny matmul for the attribute linears (bias folded in via a length column),
softmax / log-softmax epilogue, and per-core loss partials that the host
combines into the scalar loss.
"""

import os
import sys

if "/opt/trn_rl_repo" not in sys.path:
    sys.path.insert(0, "/opt/trn_rl_repo")

import numpy as np

N_CORES = 8
B, L, V, D = 16384, 50, 100000, 64
BL = B // N_CORES          # 2048 users per core
P = 128                    # partitions
NT = BL // P               # 16 tiles per core
ATTR = (2, 6, 8)
NA = 16                    # sum(ATTR)

_CACHE = {}


def _build_nc():
    import concourse.bass as bass
    import concourse.tile as tile
    from concourse import mybir
    from concourse.bacc import Bacc
    from concourse.masks import make_identity

    f32 = mybir.dt.float32
    i32 = mybir.dt.int32
    Alu = mybir.AluOpType
    Ax = mybir.AxisListType
    Act = mybir.ActivationFunctionType

    nc = Bacc(None, target_bir_lowering=False)

    x_d = nc.dram_tensor("x", [BL, L], i32, kind="ExternalInput")
    mask_d = nc.dram_tensor("mask", [BL, L], f32, kind="ExternalInput")
    y_d = nc.dram_tensor("y", [BL, NA], f32, kind="ExternalInput")
    ob_d = nc.dram_tensor("ob", [BL, NA], f32, kind="ExternalInput")
    emb_d = nc.dram_tensor("emb", [V, D], f32, kind="ExternalInput")
    wbT_d = nc.dram_tensor("wbT", [D + 1, NA], f32, kind="ExternalInput")
    logit_d = nc.dram_tensor("logit", [BL, NA], f32, kind="ExternalOutput")
    part_d = nc.dram_tensor("partials", [1, 8], f32, kind="ExternalOutput")

    with tile.TileContext(nc) as tc:
        with (
            tc.tile_pool(name="const", bufs=1) as constp,
            tc.tile_pool(name="embp", bufs=3) as embp,
            tc.tile_pool(name="work", bufs=3) as workp,
            tc.tile_pool(name="epil", bufs=1) as epil,
            tc.tile_pool(name="pst", bufs=2, space="PSUM") as pst,
            tc.tile_pool(name="psw", bufs=1, space="PSUM") as psw,
        ):
            identity = constp.tile([P, P], f32)
            make_identity(nc, identity[:])

            wbT = constp.tile([D + 1, NA], f32)
            nc.sync.dma_start(out=wbT[:], in_=wbT_d[:])

            x_sb = constp.tile([P, NT, L], i32)
            nc.sync.dma_start(out=x_sb[:], in_=x_d.rearrange("(t p) l -> p t l", p=P))
            mask_sb = constp.tile([P, NT, L], f32)
            nc.sync.dma_start(
                out=mask_sb[:], in_=mask_d.rearrange("(t p) l -> p t l", p=P)
            )
            y_sb = constp.tile([P, NT, NA], f32)
            nc.sync.dma_start(out=y_sb[:], in_=y_d.rearrange("(t p) j -> p t j", p=P))
            ob_sb = constp.tile([P, NT, NA], f32)
            nc.sync.dma_start(out=ob_sb[:], in_=ob_d.rearrange("(t p) j -> p t j", p=P))

            ones = constp.tile([P, 1], f32)
            nc.vector.memset(ones[:], 1.0)

            lens = epil.tile([P, NT, 1], f32)
            rlen = epil.tile([P, NT, 1], f32)
            W_ps = psw.tile([P, NT, NA], f32)

            for t in range(NT):
                embt = embp.tile([P, L * D], f32)
                nc.gpsimd.indirect_dma_start(
                    out=embt[:],
                    out_offset=None,
                    in_=emb_d[:],
                    in_offset=bass.IndirectOffsetOnAxis(ap=x_sb[:, t, :], axis=0),
                )
                s65 = workp.tile([P, D + 1], f32)
                # sum over history dim: view [p, (l d)] as [p, d, l], reduce l
                nc.vector.tensor_reduce(
                    out=s65[:, 0:D],
                    in_=embt[:].rearrange("p (l d) -> p d l", l=L),
                    axis=Ax.X,
                    op=Alu.add,
                )
                nc.vector.tensor_reduce(
                    out=lens[:, t, :], in_=mask_sb[:, t, :], axis=Ax.X, op=Alu.add
                )
                nc.vector.tensor_copy(out=s65[:, D : D + 1], in_=lens[:, t, :])
                tp = pst.tile([D + 1, P], f32)
                nc.tensor.transpose(out=tp[:], in_=s65[:], identity=identity[:])
                urT = workp.tile([D + 1, P], f32)
                nc.scalar.copy(out=urT[:], in_=tp[:])
                # [P,16] = urT.T @ wbT ; row 64 of wbT is the bias, scaled by len
                nc.tensor.matmul(
                    W_ps[:, t, :], urT[:], wbT[:], start=True, stop=True
                )

            # ---- epilogue over all 2048 users: [P, NT, 16] ----
            nc.vector.reciprocal(out=rlen[:], in_=lens[:])
            W = epil.tile([P, NT, NA], f32)
            nc.vector.tensor_tensor(
                out=W[:], in0=W_ps[:], in1=rlen[:].to_broadcast([P, NT, NA]),
                op=Alu.mult,
            )

            S = epil.tile([P, NT, NA], f32)   # W - max, later log-softmax
            E = epil.tile([P, NT, NA], f32)   # exp(S)
            LG = epil.tile([P, NT, NA], f32)  # softmax out
            yob = epil.tile([P, NT, NA], f32)
            nc.vector.tensor_tensor(
                out=yob[:], in0=y_sb[:], in1=ob_sb[:], op=Alu.mult
            )
            part = epil.tile([P, 8], f32)
            nc.vector.memset(part[:], 0.0)

            s = 0
            for g, w in enumerate(ATTR):
                sl = slice(s, s + w)
                mx = epil.tile([P, NT, 1], f32, tag=f"mx{g}")
                nc.vector.tensor_reduce(
                    out=mx[:], in_=W[:, :, sl], axis=Ax.X, op=Alu.max
                )
                nc.vector.tensor_tensor(
                    out=S[:, :, sl], in0=W[:, :, sl],
                    in1=mx[:].to_broadcast([P, NT, w]), op=Alu.subtract,
                )
                s += w

            nc.scalar.activation(out=E[:], in_=S[:], func=Act.Exp)

            s = 0
            for g, w in enumerate(ATTR):
                sl = slice(s, s + w)
                se = epil.tile([P, NT, 1], f32, tag=f"se{g}")
                nc.vector.tensor_reduce(
                    out=se[:], in_=E[:, :, sl], axis=Ax.X, op=Alu.add
                )
                rse = epil.tile([P, NT, 1], f32, tag=f"rse{g}")
                nc.vector.reciprocal(out=rse[:], in_=se[:])
                nc.vector.tensor_tensor(
                    out=LG[:, :, sl], in0=E[:, :, sl],
                    in1=rse[:].to_broadcast([P, NT, w]), op=Alu.mult,
                )
                lse = epil.tile([P, NT, 1], f32, tag=f"lse{g}")
                nc.scalar.activation(out=lse[:], in_=se[:], func=Act.Ln)
                # S <- log-softmax
                nc.vector.tensor_tensor(
                    out=S[:, :, sl], in0=S[:, :, sl],
                    in1=lse[:].to_broadcast([P, NT, w]), op=Alu.subtract,
                )
                C = epil.tile([P, NT, w], f32, tag=f"C{g}")
                nc.vector.tensor_tensor(
                    out=C[:], in0=yob[:, :, sl], in1=S[:, :, sl], op=Alu.mult
                )
                nc.vector.tensor_reduce(
                    out=part[:, g : g + 1], in_=C[:], axis=Ax.XY, op=Alu.add
                )
                om = epil.tile([P, NT, 1], f32, tag=f"om{g}")
                nc.vector.tensor_reduce(
                    out=om[:], in_=ob_sb[:, :, sl], axis=Ax.X, op=Alu.max
                )
                nc.vector.tensor_reduce(
                    out=part[:, 4 + g : 5 + g], in_=om[:], axis=Ax.XY, op=Alu.add
                )
                s += w

            red = pst.tile([1, 8], f32, tag="red")
            nc.tensor.matmul(red[:], ones[:], part[:], start=True, stop=True)
            out_sb = epil.tile([1, 8], f32)
            nc.scalar.copy(out=out_sb[:], in_=red[:])
            nc.sync.dma_start(out=part_d[:], in_=out_sb[:])
            nc.sync.dma_start(
                out=logit_d.rearrange("(t p) j -> p t j", p=P), in_=LG[:]
            )

    nc.compile()
    return nc


def _get_nc():
    if "nc" not in _CACHE:
        _CACHE["nc"] = _build_nc()
    return _CACHE["nc"]


def _install_trace_shim():
    """antenv.axon_hooks is absent in this container; recreate it so
    run_bass_kernel_spmd(trace=True) can capture an NTFF profile."""
    import types

    import antenv

    if "antenv.axon_hooks" in sys.modules:
        return
    mod = types.ModuleType("antenv.axon_hooks")
    hook = [None]
    mod.set_axon_ntff_profile_hook = lambda h: hook.__setitem__(0, h)
    mod.get_axon_ntff_profile_hook = lambda: hook[0]
    sys.modules["antenv.axon_hooks"] = mod
    antenv.axon_hooks = mod
    if "/root/.axon_site" not in sys.path:
        sys.path.insert(0, "/root/.axon_site")
    try:
        from trn_agent_boot.trn_boot import _ntff_profile_via_ctypes

        mod.set_axon_ntff_profile_hook(
            _ntff_profile_via_ctypes("/opt/axon/libaxon_pjrt.so")
        )
    except Exception:
        pass


def kernel(**inputs):
    from concourse.bass_utils import run_bass_kernel_spmd

    x = np.asarray(inputs["x"]).astype(np.int32).reshape(N_CORES, BL, L)
    mask = np.asarray(inputs["x_mask"]).astype(np.float32).reshape(N_CORES, BL, L)
    y = np.asarray(inputs["y"]).astype(np.float32).reshape(N_CORES, BL, NA)
    ob = np.asarray(inputs["ob"]).astype(np.float32).reshape(N_CORES, BL, NA)
    emb = np.ascontiguousarray(np.asarray(inputs["item_emb"], dtype=np.float32))
    wcat = np.concatenate(
        [np.asarray(inputs[f"w{i}"], dtype=np.float32) for i in range(3)], axis=0
    )  # [16, 64]
    bcat = np.concatenate(
        [np.asarray(inputs[f"b{i}"], dtype=np.float32) for i in range(3)], axis=0
    )  # [16]
    wbT = np.ascontiguousarray(
        np.concatenate([wcat.T, bcat[None, :]], axis=0).astype(np.float32)
    )  # [65, 16]

    in_maps = [
        {
            "x": np.ascontiguousarray(x[c]),
            "mask": np.ascontiguousarray(mask[c]),
            "y": np.ascontiguousarray(y[c]),
            "ob": np.ascontiguousarray(ob[c]),
            "emb": emb,
            "wbT": wbT,
        }
        for c in range(N_CORES)
    ]

    nc = _get_nc()
    trace = os.environ.get("KERNEL_TRACE") == "1"
    if trace:
        _install_trace_shim()
    res = run_bass_kernel_spmd(
        nc, in_maps, core_ids=list(range(N_CORES)), trace=trace
    )
    if trace:
        _CACHE["exec_time_ns"] = res.exec_time_ns
        _CACHE["profile_json"] = res.profile_json

    logit = np.concatenate([res.results[c]["logit"] for c in range(N_CORES)], axis=0)
    part = np.stack([res.results[c]["partials"][0] for c in range(N_CORES)])  # [8,8]
    num = part[:, 0:3].sum(axis=0)
    den = part[:, 4:7].sum(axis=0)
    loss = np.float32(0.0)
    for g in range(3):
        loss = np.float32(loss + (-num[g]) / max(den[g], np.float32(1.0)))
    return logit.astype(np.float32), np.float32(loss)
